# revision 1
# baseline (speedup 1.0000x reference)
"""DeltaNet attention TRN2 kernel (nn_DeltaNetAttention_5299989643476).

Strategy: data-parallel over batch (8 batches -> 8 NeuronCores). The
cross-batch cumulative_state scan is tiny ([H, Dh]) and is computed on the
host via an algebraic shortcut (mean over (b,l) of kv == Ksum . V
contraction), then passed to every core as a small constant tensor, so the
device program needs no collectives.

On-device, everything runs in a "transposed" layout (features on
partitions, sequence on the free dim):
  - QT/KT/VT projections: weight-stationary bf16 matmuls, fp32 PSUM accum
  - per head: kvT matmul; q-mod via tensor_scalar with per-partition cs;
    phi(x)=elu(x)+1 = relu(x)+exp(min(x,0)); causal linear attention as a
    masked A=pq@pk^T matmul; den via an all-ones stationary matmul (which
    also replicates den across partitions for the division broadcast);
    num needs V back in sequence-major layout -> PE transpose
  - output projection + residual + LayerNorm (bn_stats/bn_aggr)
"""

import numpy as np
import ml_dtypes

import concourse.bass as bass
import concourse.mybir as mybir
import concourse.tile as tile
from concourse import bacc
from concourse.bass_utils import run_bass_kernel_spmd
from concourse.masks import make_identity


def _ensure_axon_hooks():
    """This image's `antenv` lacks `axon_hooks`; if the caller's environment
    sets BASS_TRACE, run_bass_kernel_spmd would crash importing it. Register
    a no-op shim (only when absent) so tracing degrades gracefully."""
    try:
        import antenv.axon_hooks  # noqa: F401
    except ImportError:
        import sys
        import types

        import antenv

        mod = types.ModuleType("antenv.axon_hooks")
        _h = [None]
        mod.set_axon_ntff_profile_hook = lambda h: _h.__setitem__(0, h)
        mod.get_axon_ntff_profile_hook = lambda: _h[0]
        sys.modules["antenv.axon_hooks"] = mod
        antenv.axon_hooks = mod


_ensure_axon_hooks()

B, L, D, H = 8, 256, 2048, 8
DH = D // H            # 256
NB = D // 128          # 16 feature blocks of 128
LB = L // 128          # 2 sequence blocks of 128
EPS = 1e-5

F32 = mybir.dt.float32
BF16 = mybir.dt.bfloat16
AF = mybir.ActivationFunctionType
OP = mybir.AluOpType

_cache = {}


def _build(alpha: float, plain_ln: bool = False):
    nc = bacc.Bacc(
        "TRN2",
        target_bir_lowering=False,
        debug=False,
        enable_asserts=False,
        num_devices=B,
    )

    qT_d = nc.dram_tensor("qT", [D, L], BF16, kind="ExternalInput")
    kT_d = nc.dram_tensor("kT", [D, L], BF16, kind="ExternalInput")
    vT_d = nc.dram_tensor("vT", [D, L], BF16, kind="ExternalInput")
    qres_d = nc.dram_tensor("qres", [L, D], F32, kind="ExternalInput")
    wqT_d = nc.dram_tensor("wqT", [D, D], BF16, kind="ExternalInput")
    wkT_d = nc.dram_tensor("wkT", [D, D], BF16, kind="ExternalInput")
    wvT_d = nc.dram_tensor("wvT", [D, D], BF16, kind="ExternalInput")
    woT_d = nc.dram_tensor("woT", [D, D], BF16, kind="ExternalInput")
    csp_d = nc.dram_tensor("csp", [128, H * 2], F32, kind="ExternalInput")
    maskT_d = nc.dram_tensor("maskT", [L, L], BF16, kind="ExternalInput")
    lng_d = nc.dram_tensor("lng", [D], F32, kind="ExternalInput")
    lnb_d = nc.dram_tensor("lnb", [D], F32, kind="ExternalInput")
    out_d = nc.dram_tensor("out", [L, D], F32, kind="ExternalOutput")

    with tile.TileContext(nc) as tc:
        _body(
            tc, alpha,
            qT_d, kT_d, vT_d, qres_d,
            wqT_d, wkT_d, wvT_d, woT_d,
            csp_d, maskT_d, lng_d, lnb_d, out_d,
            plain_ln,
        )
    nc.compile()
    return nc


def _body(tc, alpha, qT_d, kT_d, vT_d, qres_d, wqT_d, wkT_d, wvT_d, woT_d,
          csp_d, maskT_d, lng_d, lnb_d, out_d, plain_ln):
    nc = tc.nc

    with (
        tc.tile_pool(name="singles", bufs=1) as singles,
        tc.tile_pool(name="wpool", bufs=6) as wpool,
        tc.tile_pool(name="big", bufs=1) as big,
        tc.tile_pool(name="hgrp", bufs=2) as hgrp,
        tc.tile_pool(name="small", bufs=3) as small,
        # one shared 4-deep PSUM pool for projections + kv: their lifetimes
        # are mostly disjoint, so sharing slots doubles each phase's
        # pipelining depth within the same 8-bank budget
        tc.tile_pool(name="psA", bufs=4, space="PSUM") as psA,
        tc.tile_pool(name="an_ps", bufs=2, space="PSUM") as an_ps,
        tc.tile_pool(name="dv_ps", bufs=2, space="PSUM") as dv_ps,
    ):
        # ---- projections: XT[i, l] = sum_j WT[j, i] * xT[j, l] ----
        # K first (pk depends only on K), then V (kv + transposes), then Q.
        # Inputs stream on the gpsimd queue, weights on the sync queue, so
        # their issue costs overlap. The K input DMA goes first on gpsimd.
        xT_in = {}
        for name, dram in (("k", kT_d), ("v", vT_d), ("q", qT_d)):
            t = big.tile([128, NB, L], BF16, tag=f"{name}T_in", name=f"{name}T_in")
            xT_in[name] = (t, dram)

        def load_xT(name):
            t, dram = xT_in[name]
            r = dram.rearrange("(n p) l -> p n l", p=128)
            # halves so the first j-blocks unblock matmuls sooner; the K
            # second half rides the sync queue ahead of the weight panels
            # (gpsimd's SWDGE drains delayed it ~5us behind the matmuls)
            nc.gpsimd.dma_start(out=t[:, 0:8, :], in_=r[:, 0:8, :])
            eng2 = nc.sync if name == "k" else nc.gpsimd
            eng2.dma_start(out=t[:, 8:16, :], in_=r[:, 8:16, :])

        load_xT("k")

        # constants after the K input on the gpsimd queue
        ident = singles.tile([128, 128], BF16)
        make_identity(nc, ident)
        ones_t = singles.tile([128, 128], BF16)
        nc.vector.memset(ones_t, 1.0)
        eps_t = singles.tile([128, 1], F32)
        nc.vector.memset(eps_t, EPS)
        csp_t = singles.tile([128, H * 2], F32)
        nc.gpsimd.dma_start(out=csp_t, in_=csp_d.ap())

        # dummy matmuls while the first weight panels stream in: keeps the
        # PE-HAM activity monitor busy so the real stream starts at 2.4 GHz
        warm_ps = dv_ps.tile([128, 256], F32, tag="dv", name="warm_ps")
        for _ in range(24):
            nc.tensor.matmul(warm_ps[:, 0:128], ones_t, ones_t,
                             start=True, stop=True)

        w_rs = {
            "k": wkT_d.rearrange("(n p) i -> p n i", p=128),
            "v": wvT_d.rearrange("(n p) i -> p n i", p=128),
            "q": wqT_d.rearrange("(n p) i -> p n i", p=128),
            "o": woT_d.rearrange("(n p) i -> p n i", p=128),
        }
        succ = {"k": "v", "v": "q", "q": "o"}
        prefetched = {}

        def panel_dma(name, iq, tag, halved=False):
            w_t = wpool.tile([128, NB, 256], BF16, tag=tag, name=f"w_{name}{iq}")
            wsl = slice(iq * 256, (iq + 1) * 256)
            w_r = w_rs[name]
            # alternate issue queues during the projections (ScalarE is idle
            # there) so issue latency and transfers overlap; outproj panels
            # stay on sync (ScalarE has real work by then)
            eng = nc.scalar if (name != "o" and iq % 2 == 1) else nc.sync
            if halved:
                eng.dma_start(out=w_t[:, 0:8, :], in_=w_r[:, 0:8, wsl])
                eng.dma_start(out=w_t[:, 8:16, :], in_=w_r[:, 8:16, wsl])
            else:
                eng.dma_start(out=w_t, in_=w_r[:, :, wsl])
            return w_t

        projs = {}
        for name in ("k", "v", "q"):
            out_t = big.tile([128, NB, L], BF16, tag=f"{name}proj",
                             name=f"{name}proj")
            x_t = xT_in[name][0]
            for iq in range(8):  # i-quarter: 2 output feature blocks
                w_t = prefetched.pop((name, iq), None)
                if w_t is None:
                    w_t = panel_dma(name, iq, "w", halved=(name == "k" and iq < 4))
                if iq == 3 and succ[name] != "o":
                    # next projection's activation streams during this proj
                    load_xT(succ[name])
                ps = psA.tile([128, 2, L], F32, tag="pk")
                for ib in range(2):
                    for j in range(NB):
                        nc.tensor.matmul(
                            ps[:, ib, :],
                            w_t[:, j, ib * 128:(ib + 1) * 128],
                            x_t[:, j, :],
                            start=(j == 0),
                            stop=(j == NB - 1),
                        )
                nc.vector.tensor_copy(out_t[:, iq * 2:iq * 2 + 2, :], ps)
            projs[name] = out_t
        KT_t, VT_t, QT_t = projs["k"], projs["v"], projs["q"]

        maskT_t = singles.tile([128, LB, L], BF16)
        nc.gpsimd.dma_start(out=maskT_t,
                            in_=maskT_d.rearrange("(a p) l -> p a l", p=128))
        qres_t = []
        for lb in range(LB):
            t = big.tile([128, D], F32, tag=f"qres{lb}", name=f"qres{lb}")
            nc.gpsimd.dma_start(out=t, in_=qres_d.ap()[lb * 128:(lb + 1) * 128, :])
            qres_t.append(t)
        lng_t = lnb_t = None
        if not plain_ln:
            lng_t = singles.tile([128, D], F32)
            nc.gpsimd.dma_start(out=lng_t,
                                in_=lng_d.ap().partition_broadcast(128))
            lnb_t = singles.tile([128, D], F32)
            nc.gpsimd.dma_start(out=lnb_t,
                                in_=lnb_d.ap().partition_broadcast(128))

        # ---- pk = phi(KT) over all heads at once ----
        pk_t = big.tile([128, NB, L], BF16, tag="pk")
        ek_t = big.tile([128, NB, L], BF16, tag="ek")
        nc.vector.tensor_scalar_min(ek_t, KT_t, 0.0)
        nc.scalar.activation(ek_t, ek_t, AF.Exp)
        nc.vector.tensor_scalar_max(pk_t, KT_t, 0.0)
        nc.vector.tensor_add(pk_t, pk_t, ek_t)

        # ---- per-head-group (2 heads): kv + V-transpose + q-mod + phi(q) ----
        # V-transposes ride along per group so PE has filler work while the
        # group's phi chain runs on DVE/ACT.
        V_t = big.tile([128, LB, D], BF16, tag="V")
        pq_t = big.tile([128, NB, L], BF16, tag="pq")
        for g in range(4):  # groups of 2 heads
            kvm = hgrp.tile([128, 4, L], BF16, tag="kvm")
            for hh in range(2):
                h = 2 * g + hh
                n0 = 2 * h
                ps = psA.tile([128, 2, L], F32, tag="pk")
                for mb in range(2):
                    for db in range(2):
                        nc.tensor.matmul(
                            ps[:, mb, :],
                            VT_t[:, n0 + db, mb * 128:(mb + 1) * 128],
                            KT_t[:, n0 + db, :],
                            start=(db == 0),
                            stop=(db == 1),
                        )
                for mb in range(2):
                    # q_mod = (alpha*Q) * (kv + cs*(1-alpha)/alpha); the
                    # alpha factor is folded into Wq on the host, so one STT
                    # straight from PSUM does modulate+multiply.
                    nc.vector.scalar_tensor_tensor(
                        out=kvm[:, 2 * hh + mb, :],
                        in0=ps[:, mb, :],
                        scalar=csp_t[:, n0 + mb:n0 + mb + 1],
                        in1=QT_t[:, n0 + mb, :],
                        op0=OP.add,
                        op1=OP.mult,
                    )
                for ib in range(LB):
                    psv = dv_ps.tile([128, 256], BF16, tag="dv")
                    for db in range(2):
                        nc.tensor.transpose(
                            psv[:, db * 128:(db + 1) * 128],
                            VT_t[:, n0 + db, ib * 128:(ib + 1) * 128],
                            ident,
                        )
                    nc.scalar.copy(
                        out=V_t[:, ib, h * DH:h * DH + 256], in_=psv[:, :]
                    )
            # pq = phi(q_mod)
            qsl = slice(4 * g, 4 * g + 4)
            eq = hgrp.tile([128, 4, L], BF16, tag="eq")
            nc.vector.tensor_scalar_min(eq, kvm, 0.0)
            nc.scalar.activation(eq, eq, AF.Exp)
            nc.vector.tensor_scalar_max(kvm, kvm, 0.0)
            nc.vector.tensor_add(pq_t[:, qsl, :], eq, kvm)


        # ---- per-head: A matmul, mask, den, num, outT ----
        attnT_t = big.tile([128, NB, L], BF16, tag="attnT")
        for h in range(H):
            n0 = 2 * h
            # causal block structure of AT[i, l] (i<=l kept):
            #   ib=0: l<128 lower-triangular, l>=128 all-ones
            #   ib=1: l<128 all-zero (skipped entirely), l>=128 triangular
            a_ps = an_ps.tile([128, 2, L], F32, tag="an")
            for db in range(2):
                nc.tensor.matmul(
                    a_ps[:, 0, :],
                    pk_t[:, n0 + db, 0:128],
                    pq_t[:, n0 + db, :],
                    start=(db == 0), stop=(db == 1),
                )
            for db in range(2):
                nc.tensor.matmul(
                    a_ps[:, 1, 128:L],
                    pk_t[:, n0 + db, 128:L],
                    pq_t[:, n0 + db, 128:L],
                    start=(db == 0), stop=(db == 1),
                )
            am = small.tile([128, LB, L], BF16, tag="am")
            nc.vector.tensor_mul(am[:, 0, 0:128], a_ps[:, 0, 0:128],
                                 maskT_t[:, 0, 0:128])
            nc.scalar.copy(out=am[:, 0, 128:L], in_=a_ps[:, 0, 128:L])
            nc.vector.tensor_mul(am[:, 1, 128:L], a_ps[:, 1, 128:L],
                                 maskT_t[:, 1, 128:L])

            den_ps = dv_ps.tile([128, L], F32, tag="dv", name="den_ps")
            nc.tensor.matmul(den_ps[:, 0:128], ones_t, am[:, 0, 0:128],
                             start=True, stop=True)
            nc.tensor.matmul(den_ps[:, 128:L], ones_t, am[:, 0, 128:L],
                             start=True, stop=False)
            nc.tensor.matmul(den_ps[:, 128:L], ones_t, am[:, 1, 128:L],
                             start=False, stop=True)
            # den is a sum of strictly positive phi-products (>= O(0.01)
            # mathematically, O(100) in practice), so the reference's 1e-8
            # clamp can never bind — reciprocal straight from PSUM.
            rden = small.tile([128, L], F32, tag="rden")
            nc.vector.reciprocal_approx_fast(out=rden, in_=den_ps)

            n_ps = an_ps.tile([128, 2, L], F32, tag="an")
            for db in range(2):
                v0 = V_t[:, 0, h * DH + db * 128:h * DH + (db + 1) * 128]
                v1 = V_t[:, 1, h * DH + db * 128:h * DH + (db + 1) * 128]
                nc.tensor.matmul(n_ps[:, db, 0:128], v0, am[:, 0, 0:128],
                                 start=True, stop=True)
                nc.tensor.matmul(n_ps[:, db, 128:L], v0, am[:, 0, 128:L],
                                 start=True, stop=False)
                nc.tensor.matmul(n_ps[:, db, 128:L], v1, am[:, 1, 128:L],
                                 start=False, stop=True)
            for db in range(2):
                nc.vector.tensor_mul(attnT_t[:, n0 + db, :], n_ps[:, db, :], rden)

        # Trigger the sqrt ACT-table load now — after ScalarE's last
        # Copy/Exp user, off the LN tail's critical path (the set switch
        # costs ~2.6us).
        warm_sqrt = singles.tile([128, 1], F32)
        nc.scalar.activation(warm_sqrt, eps_t, AF.Sqrt)

        # ---- output projection + residual + LayerNorm ----
        # 512-wide moving operand (bf16 allows 1024): half the matmul and
        # LDWEIGHTS count of the input projections. Panels are j-halves so
        # the tile stays the same size as the projection panels.
        x_sb = [big.tile([128, D], F32, tag=f"x{lb}", name=f"x{lb}")
                for lb in range(LB)]
        stats = [small.tile([128, 4, 6], F32, tag=f"stats{lb}",
                            name=f"stats{lb}", bufs=1) for lb in range(LB)]
        for nq in range(4):
            wo = []
            for jh in range(2):
                w_t = wpool.tile([128, 8, 512], BF16, tag="w",
                                 name=f"w_o{nq}{jh}")
                nc.sync.dma_start(
                    out=w_t,
                    in_=w_rs["o"][:, jh * 8:(jh + 1) * 8,
                                  nq * 512:(nq + 1) * 512])
                wo.append(w_t)
            for lb in range(LB):
                ps = psA.tile([128, 2, L], F32, tag="pk")
                psf = ps.rearrange("p a l -> p (a l)")
                for j in range(NB):
                    nc.tensor.matmul(
                        psf,
                        attnT_t[:, j, lb * 128:(lb + 1) * 128],
                        wo[j // 8][:, j % 8, :],
                        start=(j == 0),
                        stop=(j == NB - 1),
                    )
                # x = o + (query + bo)
                sl = slice(nq * 512, (nq + 1) * 512)
                nc.vector.tensor_add(x_sb[lb][:, sl], psf, qres_t[lb][:, sl])
                # LN stats pipelined per 512-chunk while later chunks project
                nc.vector.bn_stats(out=stats[lb][:, nq, :],
                                   in_=x_sb[lb][:, sl])

        for lb in range(LB):
            x = x_sb[lb]
            mv = small.tile([128, 2], F32, tag="mv")
            nc.vector.bn_aggr(out=mv, in_=stats[lb])
            sd = small.tile([128, 1], F32, tag="sd")
            nc.scalar.activation(sd, mv[:, 1:2], AF.Sqrt, bias=eps_t)
            nc.vector.reciprocal_approx_fast(out=sd, in_=sd)
            nsdmu = small.tile([128, 1], F32, tag="nsdmu")
            nc.vector.tensor_scalar(
                out=nsdmu, in0=sd, scalar1=mv[:, 0:1], scalar2=-1.0,
                op0=OP.mult, op1=OP.mult,
            )
            for ch in range(4):  # quarters, so DVE work overlaps output DMA
                sl = slice(ch * (D // 4), (ch + 1) * (D // 4))
                if plain_ln:
                    # ln_g == 1, ln_b == 0: fused (x - mu) * rstd, split
                    # across DVE and the idle ScalarE (as rstd*x - rstd*mu)
                    if ch % 2 == 0:
                        nc.vector.tensor_scalar(
                            out=x[:, sl], in0=x[:, sl], scalar1=mv[:, 0:1],
                            scalar2=sd, op0=OP.subtract, op1=OP.mult,
                        )
                    else:
                        nc.scalar.activation(
                            out=x[:, sl], in_=x[:, sl], func=AF.Identity,
                            bias=nsdmu, scale=sd,
                        )
                else:
                    nc.vector.tensor_scalar(
                        out=x[:, sl], in0=x[:, sl], scalar1=mv[:, 0:1],
                        scalar2=None, op0=OP.subtract,
                    )
                    nc.vector.scalar_tensor_tensor(
                        out=x[:, sl], in0=x[:, sl], scalar=sd, in1=lng_t[:, sl],
                        op0=OP.mult, op1=OP.mult,
                    )
                    nc.vector.tensor_add(x[:, sl], x[:, sl], lnb_t[:, sl])
                # alternate output-DMA issue queues so the ~1.2us issue
                # costs overlap at the tail
                oeng = nc.sync if ch % 2 == 0 else nc.gpsimd
                oeng.dma_start(
                    out=out_d.ap()[lb * 128:(lb + 1) * 128, sl], in_=x[:, sl])


def _host_prep(query, key, value, Wq, Wk, Wv, Wo, bo, ln_g, ln_b, alpha, beta):
    """Host-side: cumulative_state shortcut + layout/dtype marshaling."""
    a, b = float(alpha), float(beta)
    f64 = np.float64
    # mean over (batch, l) of kv[b,h,l,m] = (1/(B*L)) sum_b Ksum[b,h,:].V[b,h,m,:]
    keysum = key.astype(f64).sum(axis=1)                      # [B, D]
    Ksum = (keysum @ Wk.T.astype(f64)).reshape(B, H, DH)      # [B, H, DH]
    WvH = Wv.astype(f64).reshape(H, DH, D)
    wv_eff = np.einsum("hdj,bhd->bhj", WvH, Ksum, optimize=True)      # [B,H,D]
    contrib = np.einsum("bmj,bhj->hm", value.astype(f64), wv_eff, optimize=True)
    mean_kv = contrib / (B * L)                               # [H, DH]
    cs = np.zeros((H, DH), f64)
    c = np.zeros(DH, f64)
    for h in range(H):
        cs[h] = c
        c = b * c + a * mean_kv[h]
    # q_mod = Q*((1-a)*cs + a*kv) = (a*Q)*(kv + (1-a)/a*cs); a is folded
    # into Wq below, and this is cs*(1-a)/a:
    csp = ((1.0 - a) / a * cs if a != 0 else 0.0 * cs).astype(np.float32)
    csp_dev = np.ascontiguousarray(
        csp.reshape(H, 2, 128).transpose(2, 0, 1).reshape(128, H * 2)
    )
    plain_ln = bool(np.all(ln_g == 1.0) and np.all(ln_b == 0.0))

    bf = ml_dtypes.bfloat16
    qT = np.ascontiguousarray(query.transpose(0, 2, 1)).astype(bf)
    kT = np.ascontiguousarray(key.transpose(0, 2, 1)).astype(bf)
    vT = np.ascontiguousarray(value.transpose(0, 2, 1)).astype(bf)
    wqT = np.ascontiguousarray(a * Wq.T).astype(bf)
    wkT = np.ascontiguousarray(Wk.T).astype(bf)
    wvT = np.ascontiguousarray(Wv.T).astype(bf)
    woT = np.ascontiguousarray(Wo.T).astype(bf)
    qres = (query + bo[None, None, :]).astype(np.float32)
    maskT = np.triu(np.ones((L, L), np.float32)).astype(bf)   # maskT[i,l]=1 iff i<=l

    in_maps = []
    for c_ in range(B):
        in_maps.append({
            "qT": qT[c_], "kT": kT[c_], "vT": vT[c_],
            "qres": qres[c_],
            "wqT": wqT, "wkT": wkT, "wvT": wvT, "woT": woT,
            "csp": csp_dev, "maskT": maskT,
            "lng": ln_g.astype(np.float32), "lnb": ln_b.astype(np.float32),
        })
    return in_maps, a, plain_ln


def get_nc(alpha: float, plain_ln: bool = True):
    key = (round(float(alpha), 9), bool(plain_ln))
    if key not in _cache:
        _cache[key] = _build(float(alpha), bool(plain_ln))
    return _cache[key]


def kernel(query, key, value, Wq, Wk, Wv, Wo, bo, ln_g, ln_b, alpha, beta,
           _trace=False, _trace_kwargs=None):
    args = [np.asarray(x) for x in
            (query, key, value, Wq, Wk, Wv, Wo, bo, ln_g, ln_b, alpha, beta)]
    in_maps, a, plain_ln = _host_prep(*args)
    nc = get_nc(a, plain_ln)
    res = run_bass_kernel_spmd(
        nc, in_maps, core_ids=list(range(B)),
        trace=_trace, **(_trace_kwargs or {}),
    )
    out = np.stack([res.results[c]["out"] for c in range(B)], axis=0)
    if _trace:
        kernel._last_results = res
    return out



# revision 4
# speedup vs baseline: 1.3756x; 1.3756x over previous
"""DeltaNet attention TRN2 kernel (nn_DeltaNetAttention_5299989643476).

Strategy: data-parallel over batch (8 batches -> 8 NeuronCores). The
cross-batch cumulative_state scan is tiny ([H, Dh]) and is computed on the
host via an algebraic shortcut (mean over (b,l) of kv == Ksum . V
contraction), then passed to every core as a small constant tensor, so the
device program needs no collectives.

All four D x D projections run as fp8(e4m3) DoubleRow matmuls (2 fp8
weights per PE cell -> 2x matmul throughput and half the weight DMA
traffic). Quantization scales (activations x16, weights x512) are powers
of two and are folded downstream for free:
  - projections stay *scaled* in SBUF (bf16 exponent shift, no precision
    loss); kv/q-mod/num propagate the scale in fp32/bf16
  - phi(x)=relu(x)+exp(min(x,0)): the descale rides the Exp activation's
    `scale` input and the final add becomes an STT with the same factor
  - the den matmul's all-ones stationary becomes S/16, so rden yields
    attn pre-scaled x16 for the fp8 out-projection input
  - the out-projection STT does (psum * 2^-13) + residual
Host pre-shuffles weights/activations into partition-major DRAM layouts so
every big DMA is contiguous (or cleanly 2D) per partition.

Everything runs in a "transposed" layout (features on partitions):
  - per head: kvT matmul; q-mod via scalar_tensor_tensor with per-partition
    cs; causal linear attention as a masked A=pq@pk^T matmul; den via the
    S/16-stationary matmul (which also replicates den across partitions);
    num needs V back in sequence-major layout -> PE transpose
  - output projection (fp8 DoubleRow) + residual + LayerNorm (bn_stats)
"""

import numpy as np
import ml_dtypes

import concourse.bass as bass
import concourse.mybir as mybir
import concourse.tile as tile
from concourse import bacc
from concourse.bass_utils import run_bass_kernel_spmd
from concourse.masks import make_identity


def _ensure_axon_hooks():
    """This image's `antenv` lacks `axon_hooks`; if the caller's environment
    sets BASS_TRACE, run_bass_kernel_spmd would crash importing it. Register
    a no-op shim (only when absent) so tracing degrades gracefully."""
    try:
        import antenv.axon_hooks  # noqa: F401
    except ImportError:
        import sys
        import types

        import antenv

        mod = types.ModuleType("antenv.axon_hooks")
        _h = [None]
        mod.set_axon_ntff_profile_hook = lambda h: _h.__setitem__(0, h)
        mod.get_axon_ntff_profile_hook = lambda: _h[0]
        sys.modules["antenv.axon_hooks"] = mod
        antenv.axon_hooks = mod


_ensure_axon_hooks()

B, L, D, H = 8, 256, 2048, 8
DH = D // H            # 256
NB = D // 128          # 16 feature blocks of 128
NP = 4                 # weight panels per projection (i-slices of 512)
LB = L // 128          # 2 sequence blocks of 128
EPS = 1e-5

SA = 16.0              # fp8 activation scale
SW = 512.0             # fp8 weight scale
S = SA * SW            # 8192 = 2^13: every projection PSUM carries this
SI = 1.0 / S           # 2^-13
S2 = S * S             # kv PSUM scale (csp pre-scaled by this on host)
S3I = 1.0 / (S * S * S)  # descale inside phi(q_mod)

F32 = mybir.dt.float32
BF16 = mybir.dt.bfloat16
FP8 = mybir.dt.float8e4
AF = mybir.ActivationFunctionType
OP = mybir.AluOpType
DR = mybir.MatmulPerfMode.DoubleRow

_cache = {}


def _build(alpha: float, plain_ln: bool = False):
    nc = bacc.Bacc(
        "TRN2",
        target_bir_lowering=False,
        debug=False,
        enable_asserts=False,
        num_devices=B,
    )

    # activations pre-shuffled to [part, j-block, l]; weights to
    # [part, panel, j-block, i-within-panel] so panel DMAs are contiguous
    qT_d = nc.dram_tensor("qT", [128, NB, L], FP8, kind="ExternalInput")
    kT_d = nc.dram_tensor("kT", [128, NB, L], FP8, kind="ExternalInput")
    vT_d = nc.dram_tensor("vT", [128, NB, L], FP8, kind="ExternalInput")
    qres_d = nc.dram_tensor("qres", [L, D], F32, kind="ExternalInput")
    wqT_d = nc.dram_tensor("wqT", [128, NP, NB, 512], FP8, kind="ExternalInput")
    wkT_d = nc.dram_tensor("wkT", [128, NP, NB, 512], FP8, kind="ExternalInput")
    wvT_d = nc.dram_tensor("wvT", [128, NP, NB, 512], FP8, kind="ExternalInput")
    woT_d = nc.dram_tensor("woT", [128, NP, NB, 512], FP8, kind="ExternalInput")
    csp_d = nc.dram_tensor("csp", [128, H * 2], F32, kind="ExternalInput")
    maskT_d = nc.dram_tensor("maskT", [L, L], BF16, kind="ExternalInput")
    lng_d = nc.dram_tensor("lng", [D], F32, kind="ExternalInput")
    lnb_d = nc.dram_tensor("lnb", [D], F32, kind="ExternalInput")
    out_d = nc.dram_tensor("out", [L, D], F32, kind="ExternalOutput")

    with tile.TileContext(nc) as tc:
        _body(
            tc, alpha,
            qT_d, kT_d, vT_d, qres_d,
            wqT_d, wkT_d, wvT_d, woT_d,
            csp_d, maskT_d, lng_d, lnb_d, out_d,
            plain_ln,
        )
    nc.compile()
    return nc


def _body(tc, alpha, qT_d, kT_d, vT_d, qres_d, wqT_d, wkT_d, wvT_d, woT_d,
          csp_d, maskT_d, lng_d, lnb_d, out_d, plain_ln):
    nc = tc.nc

    with (
        tc.tile_pool(name="singles", bufs=1) as singles,
        tc.tile_pool(name="wpool", bufs=4) as wpool,
        tc.tile_pool(name="wopool", bufs=1) as wopool,
        tc.tile_pool(name="big", bufs=1) as big,
        tc.tile_pool(name="hgrp", bufs=2) as hgrp,
        tc.tile_pool(name="small", bufs=3) as small,
        tc.tile_pool(name="psA", bufs=4, space="PSUM") as psA,
        tc.tile_pool(name="an_ps", bufs=2, space="PSUM") as an_ps,
        tc.tile_pool(name="dv_ps", bufs=2, space="PSUM") as dv_ps,
    ):
        # ---- input tiles (fp8, host pre-shuffled, contiguous per part) ----
        xT_in = {}
        for name, dram in (("k", kT_d), ("v", vT_d), ("q", qT_d)):
            t = big.tile([128, NB, L], FP8, tag=f"{name}T_in", name=f"{name}T_in")
            xT_in[name] = (t, dram)

        def load_xT(name, halved=False):
            t, dram = xT_in[name]
            if halved:
                nc.gpsimd.dma_start(out=t[:, 0:8, :], in_=dram.ap()[:, 0:8, :])
                nc.gpsimd.dma_start(out=t[:, 8:16, :], in_=dram.ap()[:, 8:16, :])
            else:
                nc.gpsimd.dma_start(out=t, in_=dram.ap())

        load_xT("k", halved=True)

        # constants after the K input on the gpsimd queue
        ident = singles.tile([128, 128], BF16)
        make_identity(nc, ident)
        # den stationary: S/SA so rden comes out as SA/(S*den) and the
        # attn tensor_mul directly yields the x16-scaled fp8 out-proj input
        sden_t = singles.tile([128, 128], BF16)
        nc.vector.memset(sden_t, S / SA)
        eps_t = singles.tile([128, 1], F32)
        nc.vector.memset(eps_t, EPS)
        csp_t = singles.tile([128, H * 2], F32)
        nc.gpsimd.dma_start(out=csp_t, in_=csp_d.ap())

        # dummy matmuls while the first weight panels stream in: keeps the
        # PE-HAM activity monitor busy so the real stream starts at 2.4 GHz
        warm_ps = dv_ps.tile([128, 256], F32, tag="dv", name="warm_ps")
        for _ in range(24):
            nc.tensor.matmul(warm_ps[:, 0:128], sden_t, sden_t,
                             start=True, stop=True)

        w_drams = {"k": wkT_d, "v": wvT_d, "q": wqT_d, "o": woT_d}
        succ = {"k": "v", "v": "q", "q": None}

        def panel_dma(name, ip, halved=False):
            w_t = wpool.tile([128, NB, 512], FP8, tag="w", name=f"w_{name}{ip}")
            src = w_drams[name].ap()[:, ip]
            if halved:
                nc.sync.dma_start(out=w_t[:, 0:8, :], in_=src[:, 0:8, :])
                nc.scalar.dma_start(out=w_t[:, 8:16, :], in_=src[:, 8:16, :])
            else:
                nc.sync.dma_start(out=w_t, in_=src)
            return w_t

        # ---- projections: fp8 DoubleRow, weight-stationary ----
        # XT[i, l] = sum_j WT[j, i] * xT[j, l]; PSUM carries the 2^13 scale.
        projs = {}
        for name in ("k", "v", "q"):
            out_t = big.tile([128, NB, L], BF16, tag=f"{name}proj",
                             name=f"{name}proj")
            x_t = xT_in[name][0]
            for ip in range(NP):
                w_t = panel_dma(name, ip, halved=(name == "k" and ip == 0))
                if ip == 1 and succ[name]:
                    load_xT(succ[name])
                for t2 in range(2):  # two psum tiles of 2 output blocks each
                    ps = psA.tile([128, 2, L], F32, tag="pk")
                    for ib in range(2):
                        isl = slice((2 * t2 + ib) * 128, (2 * t2 + ib + 1) * 128)
                        for jp in range(8):
                            nc.tensor.matmul(
                                ps[:, ib, :],
                                w_t[:, 2 * jp:2 * jp + 2, isl],
                                x_t[:, 2 * jp:2 * jp + 2, :],
                                start=(jp == 0),
                                stop=(jp == 7),
                                perf_mode=DR,
                            )
                    nc.vector.tensor_copy(
                        out_t[:, ip * 4 + t2 * 2:ip * 4 + t2 * 2 + 2, :], ps)
            projs[name] = out_t
        KT_t, VT_t, QT_t = projs["k"], projs["v"], projs["q"]

        # out-proj weights: one contiguous 4MB DMA, needed ~30us later
        wo_t = wopool.tile([128, NP * NB, 512], FP8, name="wo")
        nc.sync.dma_start(
            out=wo_t, in_=woT_d.rearrange("p q n i -> p (q n) i"))

        maskT_t = singles.tile([128, LB, L], BF16)
        nc.gpsimd.dma_start(out=maskT_t,
                            in_=maskT_d.rearrange("(a p) l -> p a l", p=128))
        qres_t = []
        for lb in range(LB):
            t = big.tile([128, D], F32, tag=f"qres{lb}", name=f"qres{lb}")
            nc.gpsimd.dma_start(out=t, in_=qres_d.ap()[lb * 128:(lb + 1) * 128, :])
            qres_t.append(t)
        lng_t = lnb_t = None
        if not plain_ln:
            lng_t = singles.tile([128, D], F32)
            nc.gpsimd.dma_start(out=lng_t,
                                in_=lng_d.ap().partition_broadcast(128))
            lnb_t = singles.tile([128, D], F32)
            nc.gpsimd.dma_start(out=lnb_t,
                                in_=lnb_d.ap().partition_broadcast(128))

        # ---- pk = phi(KT) over all heads at once (KT is S-scaled) ----
        pk_t = big.tile([128, NB, L], BF16, tag="pk")
        ek_t = big.tile([128, NB, L], BF16, tag="ek")
        nc.vector.tensor_scalar_min(ek_t, KT_t, 0.0)
        nc.scalar.activation(ek_t, ek_t, AF.Exp, scale=SI)
        nc.vector.tensor_scalar_max(pk_t, KT_t, 0.0)
        nc.vector.scalar_tensor_tensor(
            out=pk_t, in0=pk_t, scalar=SI, in1=ek_t, op0=OP.mult, op1=OP.add)

        # ---- per-head-group (2 heads): kv + V-transpose + q-mod + phi(q) ----
        # V-transposes ride along per group so PE has filler work while the
        # group's phi chain runs on DVE/ACT.
        V_t = big.tile([128, LB, D], BF16, tag="V")
        pq_t = big.tile([128, NB, L], BF16, tag="pq")
        for g in range(4):  # groups of 2 heads
            kvm = hgrp.tile([128, 4, L], BF16, tag="kvm")
            for hh in range(2):
                h = 2 * g + hh
                n0 = 2 * h
                ps = psA.tile([128, 2, L], F32, tag="pk")
                for mb in range(2):
                    for db in range(2):
                        nc.tensor.matmul(
                            ps[:, mb, :],
                            VT_t[:, n0 + db, mb * 128:(mb + 1) * 128],
                            KT_t[:, n0 + db, :],
                            start=(db == 0),
                            stop=(db == 1),
                        )
                for mb in range(2):
                    # q_mod (x S^3) = (kv_ps + cs_scaled) * QT; csp carries
                    # S^2 and the alpha fold from the host.
                    nc.vector.scalar_tensor_tensor(
                        out=kvm[:, 2 * hh + mb, :],
                        in0=ps[:, mb, :],
                        scalar=csp_t[:, n0 + mb:n0 + mb + 1],
                        in1=QT_t[:, n0 + mb, :],
                        op0=OP.add,
                        op1=OP.mult,
                    )
                for ib in range(LB):
                    psv = dv_ps.tile([128, 256], BF16, tag="dv")
                    for db in range(2):
                        nc.tensor.transpose(
                            psv[:, db * 128:(db + 1) * 128],
                            VT_t[:, n0 + db, ib * 128:(ib + 1) * 128],
                            ident,
                        )
                    nc.scalar.copy(
                        out=V_t[:, ib, h * DH:h * DH + 256], in_=psv[:, :]
                    )
            # pq = phi(q_mod): descale by S^-3 inside Exp and the STT add
            qsl = slice(4 * g, 4 * g + 4)
            eq = hgrp.tile([128, 4, L], BF16, tag="eq")
            nc.vector.tensor_scalar_min(eq, kvm, 0.0)
            nc.scalar.activation(eq, eq, AF.Exp, scale=S3I)
            nc.vector.tensor_scalar_max(kvm, kvm, 0.0)
            nc.vector.scalar_tensor_tensor(
                out=pq_t[:, qsl, :], in0=kvm, scalar=S3I, in1=eq,
                op0=OP.mult, op1=OP.add)

        # ---- per-head: A matmul, mask, den, num, attnT (fp8, x16) ----
        attnT_t = big.tile([128, NB, L], FP8, tag="attnT")
        for h in range(H):
            n0 = 2 * h
            # causal block structure of AT[i, l] (i<=l kept):
            #   ib=0: l<128 lower-triangular, l>=128 all-ones
            #   ib=1: l<128 all-zero (skipped entirely), l>=128 triangular
            a_ps = an_ps.tile([128, 2, L], F32, tag="an")
            for db in range(2):
                nc.tensor.matmul(
                    a_ps[:, 0, :],
                    pk_t[:, n0 + db, 0:128],
                    pq_t[:, n0 + db, :],
                    start=(db == 0), stop=(db == 1),
                )
            for db in range(2):
                nc.tensor.matmul(
                    a_ps[:, 1, 128:L],
                    pk_t[:, n0 + db, 128:L],
                    pq_t[:, n0 + db, 128:L],
                    start=(db == 0), stop=(db == 1),
                )
            am = small.tile([128, LB, L], BF16, tag="am")
            nc.vector.tensor_mul(am[:, 0, 0:128], a_ps[:, 0, 0:128],
                                 maskT_t[:, 0, 0:128])
            nc.scalar.copy(out=am[:, 0, 128:L], in_=a_ps[:, 0, 128:L])
            nc.vector.tensor_mul(am[:, 1, 128:L], a_ps[:, 1, 128:L],
                                 maskT_t[:, 1, 128:L])

            den_ps = dv_ps.tile([128, L], F32, tag="dv", name="den_ps")
            nc.tensor.matmul(den_ps[:, 0:128], sden_t, am[:, 0, 0:128],
                             start=True, stop=True)
            nc.tensor.matmul(den_ps[:, 128:L], sden_t, am[:, 0, 128:L],
                             start=True, stop=False)
            nc.tensor.matmul(den_ps[:, 128:L], sden_t, am[:, 1, 128:L],
                             start=False, stop=True)
            # den is a sum of strictly positive phi-products (>= O(0.01)
            # mathematically, O(100) in practice), so the reference's 1e-8
            # clamp can never bind — reciprocal straight from PSUM.
            rden = small.tile([128, L], F32, tag="rden")
            nc.vector.reciprocal_approx_fast(out=rden, in_=den_ps)

            n_ps = an_ps.tile([128, 2, L], F32, tag="an")
            for db in range(2):
                v0 = V_t[:, 0, h * DH + db * 128:h * DH + (db + 1) * 128]
                v1 = V_t[:, 1, h * DH + db * 128:h * DH + (db + 1) * 128]
                nc.tensor.matmul(n_ps[:, db, 0:128], v0, am[:, 0, 0:128],
                                 start=True, stop=True)
                nc.tensor.matmul(n_ps[:, db, 128:L], v0, am[:, 0, 128:L],
                                 start=True, stop=False)
                nc.tensor.matmul(n_ps[:, db, 128:L], v1, am[:, 1, 128:L],
                                 start=False, stop=True)
            for db in range(2):
                # n_ps carries S (V side), rden carries SA/S -> fp8 x16
                nc.vector.tensor_mul(attnT_t[:, n0 + db, :], n_ps[:, db, :], rden)

        # Trigger the sqrt ACT-table load now — after ScalarE's last
        # Copy/Exp user, off the LN tail's critical path (the set switch
        # costs ~2.6us).
        warm_sqrt = singles.tile([128, 1], F32)
        nc.scalar.activation(warm_sqrt, eps_t, AF.Sqrt)

        # ---- output projection (fp8 DoubleRow) + residual + LayerNorm ----
        # lb-outer so block 0's LN + store overlap block 1's matmuls.
        x_sb = [big.tile([128, D], F32, tag=f"x{lb}", name=f"x{lb}")
                for lb in range(LB)]
        stats = [small.tile([128, 4, 6], F32, tag=f"stats{lb}",
                            name=f"stats{lb}", bufs=1) for lb in range(LB)]
        for lb in range(LB):
            lsl = slice(lb * 128, (lb + 1) * 128)
            for nq in range(4):
                ps = psA.tile([128, 2, L], F32, tag="pk")
                for half in range(2):
                    for jp in range(8):
                        nc.tensor.matmul(
                            ps[:, half, :],
                            attnT_t[:, 2 * jp:2 * jp + 2, lsl],
                            wo_t[:, nq * NB + 2 * jp:nq * NB + 2 * jp + 2,
                                 half * 256:(half + 1) * 256],
                            start=(jp == 0),
                            stop=(jp == 7),
                            perf_mode=DR,
                        )
                psf = ps.rearrange("p a l -> p (a l)")
                # x = o + (query + bo): PSUM carries 2^13
                sl = slice(nq * 512, (nq + 1) * 512)
                nc.vector.scalar_tensor_tensor(
                    out=x_sb[lb][:, sl], in0=psf, scalar=SI,
                    in1=qres_t[lb][:, sl], op0=OP.mult, op1=OP.add)
                nc.vector.bn_stats(out=stats[lb][:, nq, :],
                                   in_=x_sb[lb][:, sl])

            x = x_sb[lb]
            mv = small.tile([128, 2], F32, tag="mv")
            nc.vector.bn_aggr(out=mv, in_=stats[lb])
            sd = small.tile([128, 1], F32, tag="sd")
            nc.scalar.activation(sd, mv[:, 1:2], AF.Sqrt, bias=eps_t)
            nc.vector.reciprocal_approx_fast(out=sd, in_=sd)
            nsdmu = small.tile([128, 1], F32, tag="nsdmu")
            nc.vector.tensor_scalar(
                out=nsdmu, in0=sd, scalar1=mv[:, 0:1], scalar2=-1.0,
                op0=OP.mult, op1=OP.mult,
            )
            for ch in range(4):  # quarters, so DVE work overlaps output DMA
                sl = slice(ch * (D // 4), (ch + 1) * (D // 4))
                if plain_ln:
                    # ln_g == 1, ln_b == 0: fused (x - mu) * rstd, split
                    # across DVE and the idle ScalarE (as rstd*x - rstd*mu)
                    if ch % 2 == 0:
                        nc.vector.tensor_scalar(
                            out=x[:, sl], in0=x[:, sl], scalar1=mv[:, 0:1],
                            scalar2=sd, op0=OP.subtract, op1=OP.mult,
                        )
                    else:
                        nc.scalar.activation(
                            out=x[:, sl], in_=x[:, sl], func=AF.Identity,
                            bias=nsdmu, scale=sd,
                        )
                else:
                    nc.vector.tensor_scalar(
                        out=x[:, sl], in0=x[:, sl], scalar1=mv[:, 0:1],
                        scalar2=None, op0=OP.subtract,
                    )
                    nc.vector.scalar_tensor_tensor(
                        out=x[:, sl], in0=x[:, sl], scalar=sd, in1=lng_t[:, sl],
                        op0=OP.mult, op1=OP.mult,
                    )
                    nc.vector.tensor_add(x[:, sl], x[:, sl], lnb_t[:, sl])
                # alternate output-DMA issue queues so the ~1.2us issue
                # costs overlap at the tail
                oeng = nc.sync if ch % 2 == 0 else nc.gpsimd
                oeng.dma_start(
                    out=out_d.ap()[lb * 128:(lb + 1) * 128, sl], in_=x[:, sl])


def _to_f8(x):
    return np.clip(x, -240.0, 240.0).astype(ml_dtypes.float8_e4m3)


def _shuffle_w(wT):
    """[D, D] (j, i) -> [128, NP, NB, 512]: panel DMAs contiguous per part."""
    return np.ascontiguousarray(
        wT.reshape(NB, 128, NP, 512).transpose(1, 2, 0, 3))


def _shuffle_x(xT):
    """[B, D, L] -> [B, 128, NB, L]: contiguous per partition."""
    Bs = xT.shape[0]
    return np.ascontiguousarray(
        xT.reshape(Bs, NB, 128, L).transpose(0, 2, 1, 3))


def _host_prep(query, key, value, Wq, Wk, Wv, Wo, bo, ln_g, ln_b, alpha, beta):
    """Host-side: cumulative_state shortcut + fp8 quant + layout marshaling."""
    a, b = float(alpha), float(beta)
    f64 = np.float64
    # mean over (batch, l) of kv[b,h,l,m] = (1/(B*L)) sum_b Ksum[b,h,:].V[b,h,m,:]
    keysum = key.astype(f64).sum(axis=1)                      # [B, D]
    Ksum = (keysum @ Wk.T.astype(f64)).reshape(B, H, DH)      # [B, H, DH]
    WvH = Wv.astype(f64).reshape(H, DH, D)
    wv_eff = np.einsum("hdj,bhd->bhj", WvH, Ksum, optimize=True)      # [B,H,D]
    contrib = np.einsum("bmj,bhj->hm", value.astype(f64), wv_eff, optimize=True)
    mean_kv = contrib / (B * L)                               # [H, DH]
    cs = np.zeros((H, DH), f64)
    c = np.zeros(DH, f64)
    for h in range(H):
        cs[h] = c
        c = b * c + a * mean_kv[h]
    # q_mod = Q*((1-a)*cs + a*kv) = (a*Q)*(kv + (1-a)/a*cs); a is folded
    # into Wq below; device kv PSUM carries S^2, so csp does too:
    csp = (S2 * (1.0 - a) / a * cs if a != 0 else 0.0 * cs).astype(np.float32)
    csp_dev = np.ascontiguousarray(
        csp.reshape(H, 2, 128).transpose(2, 0, 1).reshape(128, H * 2)
    )
    plain_ln = bool(np.all(ln_g == 1.0) and np.all(ln_b == 0.0))

    bf = ml_dtypes.bfloat16
    qT = _shuffle_x(_to_f8(SA * query.transpose(0, 2, 1)))
    kT = _shuffle_x(_to_f8(SA * key.transpose(0, 2, 1)))
    vT = _shuffle_x(_to_f8(SA * value.transpose(0, 2, 1)))
    wqT = _shuffle_w(_to_f8(SW * a * Wq.T))
    wkT = _shuffle_w(_to_f8(SW * Wk.T))
    wvT = _shuffle_w(_to_f8(SW * Wv.T))
    woT = _shuffle_w(_to_f8(SW * Wo.T))
    qres = (query + bo[None, None, :]).astype(np.float32)
    maskT = np.triu(np.ones((L, L), np.float32)).astype(bf)   # maskT[i,l]=1 iff i<=l

    in_maps = []
    for c_ in range(B):
        in_maps.append({
            "qT": qT[c_], "kT": kT[c_], "vT": vT[c_],
            "qres": qres[c_],
            "wqT": wqT, "wkT": wkT, "wvT": wvT, "woT": woT,
            "csp": csp_dev, "maskT": maskT,
            "lng": ln_g.astype(np.float32), "lnb": ln_b.astype(np.float32),
        })
    return in_maps, a, plain_ln


def get_nc(alpha: float, plain_ln: bool = True):
    key = (round(float(alpha), 9), bool(plain_ln))
    if key not in _cache:
        _cache[key] = _build(float(alpha), bool(plain_ln))
    return _cache[key]


def kernel(query, key, value, Wq, Wk, Wv, Wo, bo, ln_g, ln_b, alpha, beta,
           _trace=False, _trace_kwargs=None):
    args = [np.asarray(x) for x in
            (query, key, value, Wq, Wk, Wv, Wo, bo, ln_g, ln_b, alpha, beta)]
    in_maps, a, plain_ln = _host_prep(*args)
    nc = get_nc(a, plain_ln)
    res = run_bass_kernel_spmd(
        nc, in_maps, core_ids=list(range(B)),
        trace=_trace, **(_trace_kwargs or {}),
    )
    out = np.stack([res.results[c]["out"] for c in range(B)], axis=0)
    if _trace:
        kernel._last_results = res
    return out
